# revision 1
# baseline (speedup 1.0000x reference)
"""Causal attention (B=4, S=2048, D=1024) on 8 Trainium2 NeuronCores.

Sharding: 2 cores per batch element. Within a batch, the 8 query blocks of
256 rows are split between the two cores by parity (core fold 0 takes odd
blocks, fold 1 takes even blocks) so the causal-attention work is balanced.
Each core computes Q for its own 1024 query rows, and K/V for the full 2048
context rows (duplicated across the pair — cheaper than a collective here).

All matmuls run as float32r (FP22 multiply, FP32 accumulate) which streams at
bf16 rate on the PE for free dims >= 256 while keeping ~1e-4 relative error.

Layout trick: scores are computed transposed (k on partitions, q on free dim)
via S^T = K^T.T @ Q^T, so no on-chip transpose of the softmax matrix is ever
needed: exp(S^T) tiles feed the attn@V matmul directly as the stationary
operand, producing the output in natural [q, o] layout. Softmax is computed
without max-subtraction (scores/sqrt(d) are ~N(0,1) here, exp is safe) and
the denominator comes from an extra ones-column matmul; causal masking is a
multiply with 0/1 mask tiles streamed from the host (per-core data, so one
SPMD program serves both folds).
"""

import sys

sys.path.insert(0, "/opt/trn_rl_repo")

import numpy as np

import concourse.bass as bass  # noqa: F401  (bass must import before mybir use)
import concourse.mybir as mybir
import concourse.tile as tile
from concourse import bacc
from concourse.bass_utils import run_bass_kernel_spmd

F32 = mybir.dt.float32
F32R = mybir.dt.float32r
AF = mybir.ActivationFunctionType

B, S, D = 4, 2048, 1024
P = 128
DC = D // P  # 8 contraction chunks
OC = D // P  # 8 output-feature chunks
TC = S // P  # 16 context chunks
N_CORES = 8
SLOTS = 4  # query slots of 256 rows per core
QB = 256  # query rows per slot
# Padded causal depth (in 128-wide k tiles) per slot, shared by both folds.
# fold 0 owns global 256-blocks [1,3,5,7] (true depths 4,8,12,16),
# fold 1 owns [0,2,4,6] (true depths 2,6,10,14) -> padded to the fold-0 depths.
KT_COUNTS = [4, 8, 12, 16]
FOLD_QBLOCKS = {0: [1, 3, 5, 7], 1: [0, 2, 4, 6]}
N_MASK = sum(KT_COUNTS[s] - 4 * s for s in range(SLOTS))  # 16 mask tiles
SCALE = 1.0 / np.sqrt(np.float32(D))


def _build_nc():
    nc = bacc.Bacc("TRN2", target_bir_lowering=False, debug=False, num_devices=N_CORES)

    xT_d = nc.declare_dram_parameter("xT", [D, S], F32, isOutput=False)
    xTq_d = nc.declare_dram_parameter("xTq", [D, SLOTS * QB], F32, isOutput=False)
    wq_d = nc.declare_dram_parameter("wqT", [D, D], F32, isOutput=False)
    wk_d = nc.declare_dram_parameter("wkT", [D, D], F32, isOutput=False)
    wv_d = nc.declare_dram_parameter("wvT", [D, D], F32, isOutput=False)
    mask_d = nc.declare_dram_parameter("masks", [N_MASK, P, QB], F32, isOutput=False)
    out_d = nc.declare_dram_parameter("out", [SLOTS * QB, D], F32, isOutput=True)

    xT = xT_d[:].rearrange("(dc p) t -> p dc t", p=P)  # [128, 8, 2048]
    xTq = xTq_d[:].rearrange("(dc p) q -> p dc q", p=P)  # [128, 8, 1024]
    wq = wq_d[:].rearrange("(dc p) o -> p dc o", p=P)  # [128, 8, 1024]
    wk = wk_d[:].rearrange("(dc p) o -> p dc o", p=P)
    wv = wv_d[:].rearrange("(dc p) o -> p dc o", p=P)
    out_r = out_d[:].rearrange("(qc p) o -> p qc o", p=P)  # [128, 8, 1024]

    with tile.TileContext(nc) as tc:
        with tc.tile_pool(name="resident", bufs=1) as res_pool:
            # K^T resident: [o-part, oc, t]  (f32r so it feeds matmuls directly)
            kt_res = res_pool.tile([P, OC, S], F32R, name="kt_res")
            # V resident: [t-part, tc, o]
            v_res = res_pool.tile([P, TC, D], F32R, name="v_res")
            ones2 = res_pool.tile([P, 2], F32R, name="ones2")
            nc.vector.memset(ones2[:].bitcast(F32), 1.0)

            with tc.tile_pool(name="dram_scratch", bufs=1, space="DRAM") as dpool:
                qt_dram = dpool.tile([P, OC, SLOTS * QB], F32, name="qt_dram")

                # ---------------- Phase Q: Q^T = Wq^T.T @ xTq -> qt_dram ----
                with (
                    tc.tile_pool(name="w_pool", bufs=1) as wpool,
                    tc.tile_pool(name="x_pool", bufs=2) as xpool,
                    tc.tile_pool(name="ev_pool", bufs=3) as evpool,
                    tc.tile_pool(name="psum_q", bufs=4, space="PSUM") as pspool,
                ):
                    w_t = wpool.tile([P, DC, D], F32R, name="wq_t")
                    nc.sync.dma_start(w_t[:], wq.bitcast(F32R))
                    for qt in range(4):  # 256-wide query column tiles
                        x_t = xpool.tile([P, DC, QB], F32R, name="xq_t")
                        nc.sync.dma_start(
                            x_t[:], xTq[:, :, QB * qt : QB * (qt + 1)].bitcast(F32R)
                        )
                        for oc in range(OC):
                            ps = pspool.tile([P, QB], F32, name="ps_q")
                            for dc in range(DC):
                                nc.tensor.matmul(
                                    ps[:],
                                    lhsT=w_t[:, dc, P * oc : P * (oc + 1)],
                                    rhs=x_t[:, dc, :],
                                    start=(dc == 0),
                                    stop=(dc == DC - 1),
                                )
                            ev = evpool.tile([P, QB], F32, name="qt_ev")
                            nc.vector.tensor_copy(ev[:], ps[:])
                            nc.sync.dma_start(
                                qt_dram[:, oc, QB * qt : QB * (qt + 1)], ev[:]
                            )

                # ---------------- Phase K: K^T = Wk^T.T @ xT -> kt_res ------
                with (
                    tc.tile_pool(name="w_pool", bufs=1) as wpool,
                    tc.tile_pool(name="x_pool", bufs=2) as xpool,
                    tc.tile_pool(name="psum_k", bufs=4, space="PSUM") as pspool,
                ):
                    w_t = wpool.tile([P, DC, D], F32R, name="wk_t")
                    nc.sync.dma_start(w_t[:], wk.bitcast(F32R))
                    for tt in range(8):  # 256-wide context column tiles
                        x_t = xpool.tile([P, DC, 256], F32R, name="xk_t")
                        nc.sync.dma_start(
                            x_t[:], xT[:, :, 256 * tt : 256 * (tt + 1)].bitcast(F32R)
                        )
                        for oc in range(OC):
                            ps = pspool.tile([P, 256], F32, name="ps_k")
                            for dc in range(DC):
                                nc.tensor.matmul(
                                    ps[:],
                                    lhsT=w_t[:, dc, P * oc : P * (oc + 1)],
                                    rhs=x_t[:, dc, :],
                                    start=(dc == 0),
                                    stop=(dc == DC - 1),
                                )
                            nc.vector.tensor_copy(
                                kt_res[:, oc, 256 * tt : 256 * (tt + 1)], ps[:]
                            )

                # ---------------- Phase V: V = xT.T @ Wv^T -> v_res ---------
                with (
                    tc.tile_pool(name="w_pool", bufs=1) as wpool,
                    tc.tile_pool(name="x_pool", bufs=2) as xpool,
                    tc.tile_pool(name="psum_v", bufs=4, space="PSUM") as pspool,
                ):
                    w_t = wpool.tile([P, DC, D], F32R, name="wv_t")
                    nc.sync.dma_start(w_t[:], wv.bitcast(F32R))
                    for tt in range(8):
                        x_t = xpool.tile([P, DC, 256], F32R, name="xv_t")
                        nc.sync.dma_start(
                            x_t[:], xT[:, :, 256 * tt : 256 * (tt + 1)].bitcast(F32R)
                        )
                        for tci in range(2):  # 128-row chunks within the tile
                            tcg = 2 * tt + tci
                            for ot in range(2):  # 512-wide output feature tiles
                                ps = pspool.tile([P, 512], F32, name="ps_v")
                                for dc in range(DC):
                                    nc.tensor.matmul(
                                        ps[:],
                                        lhsT=x_t[:, dc, P * tci : P * (tci + 1)],
                                        rhs=w_t[:, dc, 512 * ot : 512 * (ot + 1)],
                                        start=(dc == 0),
                                        stop=(dc == DC - 1),
                                    )
                                nc.vector.tensor_copy(
                                    v_res[:, tcg, 512 * ot : 512 * (ot + 1)], ps[:]
                                )

            # ---------------- Phase A: attention per query slot -------------
            with (
                tc.tile_pool(name="qt_pool", bufs=2) as qpool,
                tc.tile_pool(name="es_pool", bufs=3) as epool,
                tc.tile_pool(name="mk_pool", bufs=2) as mpool,
                tc.tile_pool(name="ob_pool", bufs=3) as opool,
                tc.tile_pool(name="rc_pool", bufs=2) as rpool,
                tc.tile_pool(name="psum_s", bufs=2, space="PSUM") as pss,
                tc.tile_pool(name="psum_o", bufs=4, space="PSUM") as pso_pool,
                tc.tile_pool(name="psum_d", bufs=2, space="PSUM") as psd_pool,
            ):
                mask_i = 0
                for s in range(SLOTS):
                    qts = qpool.tile([P, OC, QB], F32R, name="qt_slot")
                    nc.sync.dma_start(
                        qts[:], qt_dram[:, :, QB * s : QB * (s + 1)].bitcast(F32R)
                    )
                    pso = [
                        [pso_pool.tile([P, 512], F32, name="ps_o") for _ in range(2)]
                        for _ in range(2)
                    ]
                    psd = [psd_pool.tile([P, 2], F32, name="ps_d") for _ in range(2)]
                    nkt = KT_COUNTS[s]
                    for kt in range(nkt):
                        ps_s = pss.tile([P, QB], F32, name="ps_s")
                        for oc in range(OC):
                            nc.tensor.matmul(
                                ps_s[:],
                                lhsT=kt_res[:, oc, P * kt : P * (kt + 1)],
                                rhs=qts[:, oc, :],
                                start=(oc == 0),
                                stop=(oc == OC - 1),
                            )
                        es = epool.tile([P, QB], F32R, name="es")
                        nc.scalar.activation(es[:], ps_s[:], AF.Exp, scale=SCALE)
                        if kt >= 4 * s:
                            mt = mpool.tile([P, QB], F32R, name="mask_t")
                            nc.sync.dma_start(mt[:], mask_d[mask_i].bitcast(F32R))
                            nc.vector.tensor_mul(out=es[:], in0=es[:], in1=mt[:])
                            mask_i += 1
                        first, last = (kt == 0), (kt == nkt - 1)
                        for qcc in range(2):
                            lhs = es[:, P * qcc : P * (qcc + 1)]
                            for ot in range(2):
                                nc.tensor.matmul(
                                    pso[qcc][ot][:],
                                    lhsT=lhs,
                                    rhs=v_res[:, kt, 512 * ot : 512 * (ot + 1)],
                                    start=first,
                                    stop=last,
                                )
                            nc.tensor.matmul(
                                psd[qcc][:],
                                lhsT=lhs,
                                rhs=ones2[:],
                                start=first,
                                stop=last,
                            )
                    for qcc in range(2):
                        rc = rpool.tile([P, 1], F32, name="rc")
                        nc.vector.reciprocal(rc[:], psd[qcc][:, 0:1])
                        for ot in range(2):
                            ob = opool.tile([P, 512], F32, name="ob")
                            nc.scalar.activation(
                                ob[:], pso[qcc][ot][:], AF.Copy, scale=rc[:]
                            )
                            nc.sync.dma_start(
                                out_r[:, 2 * s + qcc, 512 * ot : 512 * (ot + 1)], ob[:]
                            )

    nc.compile()
    if not nc.is_finalized():
        nc.finalize()
    return nc


def _build_masks(fold: int) -> np.ndarray:
    """0/1 mask tiles [N_MASK, 128, 256] for the tiles at/above the diagonal."""
    tiles = []
    ki = np.arange(P)[:, None]
    qi = np.arange(QB)[None, :]
    for s in range(SLOTS):
        qb = FOLD_QBLOCKS[fold][s]
        q0 = qb * QB
        for kt in range(4 * s, KT_COUNTS[s]):
            k0 = kt * P
            tiles.append(((q0 + qi) >= (k0 + ki)).astype(np.float32))
    return np.ascontiguousarray(np.stack(tiles))


def kernel(**inputs: np.ndarray) -> np.ndarray:
    x = np.ascontiguousarray(np.asarray(inputs["inputs"], dtype=np.float32))
    wqT = np.ascontiguousarray(np.asarray(inputs["Wq"], dtype=np.float32).T)
    wkT = np.ascontiguousarray(np.asarray(inputs["Wk"], dtype=np.float32).T)
    wvT = np.ascontiguousarray(np.asarray(inputs["Wv"], dtype=np.float32).T)

    masks = {f: _build_masks(f) for f in (0, 1)}
    in_maps = []
    for c in range(N_CORES):
        b, f = c // 2, c % 2
        xT = np.ascontiguousarray(x[b].T)  # [D, S]
        xTq = np.ascontiguousarray(
            np.concatenate(
                [xT[:, qb * QB : (qb + 1) * QB] for qb in FOLD_QBLOCKS[f]], axis=1
            )
        )
        in_maps.append(
            {
                "xT": xT,
                "xTq": xTq,
                "wqT": wqT,
                "wkT": wkT,
                "wvT": wvT,
                "masks": masks[f],
            }
        )

    nc = _build_nc()
    res = run_bass_kernel_spmd(nc, in_maps, core_ids=list(range(N_CORES)))

    out = np.empty((B, S, D), dtype=np.float32)
    for c in range(N_CORES):
        b, f = c // 2, c % 2
        o = res.results[c]["out"]  # [1024, 1024] rows in slot order
        for s, qb in enumerate(FOLD_QBLOCKS[f]):
            out[b, qb * QB : (qb + 1) * QB, :] = o[s * QB : (s + 1) * QB, :]
    return out


# revision 3
# speedup vs baseline: 27.4896x; 27.4896x over previous
"""Causal attention (B=4, S=2048, D=1024) on 8 Trainium2 NeuronCores.

Sharding: 2 cores per batch element. Within a batch, the 8 query blocks of
256 rows are split between the two cores by parity (fold 0 takes odd blocks,
fold 1 takes even blocks) so causal-attention work is balanced. Each core
computes Q for its own 1024 query rows, and K/V for the full 2048 context
rows (duplicated across the pair — cheaper than a collective here).

All matmuls run as float32r (FP22 multiply, FP32 accumulate) which streams at
bf16 rate on the PE for free dims >= 256 while keeping ~3e-4 relative error.
fp32 weights have no fast-weight-load path, so every matmul pays a ~215 ns
LDWEIGHTS; all matmuls therefore use N=512 moving operands so compute time
covers the weight load.

Layout trick: scores are computed transposed (k on partitions, q on free dim)
via S^T = K^T.T @ Q^T, so no transpose of the softmax matrix is needed:
exp(S^T) tiles feed attn@V directly as the stationary operand, producing the
output in natural [q, o] layout. Scores for two adjacent 256-row query slots
are computed together (N=512) over the union of their causal depths; the 0/1
causal masks (streamed per-core from the host, so one SPMD program serves
both folds) zero both the diagonal parts and the over-computed region, which
also makes the softmax denominators correct. Softmax skips max-subtraction
(scores/sqrt(d) are ~N(0,1) here; exp cannot overflow), with denominators
from a ones-column matmul per 128-query chunk. V lives in DRAM between its
projection and the attention phase to keep SBUF under the allocator cap.
"""

import sys

sys.path.insert(0, "/opt/trn_rl_repo")

import numpy as np

import concourse.bass as bass  # noqa: F401
import concourse.mybir as mybir
import concourse.tile as tile
from concourse import bacc
from concourse.bass_utils import run_bass_kernel_spmd

F32 = mybir.dt.float32
F32R = mybir.dt.float32r
AF = mybir.ActivationFunctionType

B, S, D = 4, 2048, 1024
P = 128
DC = D // P  # 8 contraction chunks
OC = D // P  # 8 output-feature chunks
TC = S // P  # 16 context chunks
N_CORES = 8
SLOTS = 4  # query slots of 256 rows per core
QB = 256
# Padded causal depth (in 128-wide k tiles) per slot, fold-uniform:
# fold 0 owns global 256-blocks [1,3,5,7] (true depths 4,8,12,16),
# fold 1 owns [0,2,4,6] (true depths 2,6,10,14) -> padded to fold-0 depths.
KT_COUNTS = [4, 8, 12, 16]
FOLD_QBLOCKS = {0: [1, 3, 5, 7], 1: [0, 2, 4, 6]}
# Slot pairs (0,1) and (2,3) share one N=512 scores pass over the union depth.
PAIR_DEPTH = [KT_COUNTS[1], KT_COUNTS[3]]  # [8, 16]
N_MASK = sum(PAIR_DEPTH) - 8  # pair0: kt 0..7 masked; pair1: kt 8..15 masked
SCALE = 1.0 / np.sqrt(np.float32(D))


def _build_nc(repeat: int = 1):
    nc = bacc.Bacc("TRN2", target_bir_lowering=False, debug=False, num_devices=N_CORES)

    xT_d = nc.declare_dram_parameter("xT", [D, S], F32, isOutput=False)
    xTq_d = nc.declare_dram_parameter("xTq", [D, SLOTS * QB], F32, isOutput=False)
    wq_d = nc.declare_dram_parameter("wqT", [D, D], F32, isOutput=False)
    wk_d = nc.declare_dram_parameter("wkT", [D, D], F32, isOutput=False)
    wv_d = nc.declare_dram_parameter("wvT", [D, D], F32, isOutput=False)
    mask_d = nc.declare_dram_parameter("masks", [N_MASK, P, 2 * QB], F32, isOutput=False)
    out_d = nc.declare_dram_parameter("out", [SLOTS * QB, D], F32, isOutput=True)

    xT = xT_d[:].rearrange("(dc p) t -> p dc t", p=P)  # [128, 8, 2048]
    xTq = xTq_d[:].rearrange("(dc p) q -> p dc q", p=P)  # [128, 8, 1024]
    wq = wq_d[:].rearrange("(dc p) o -> p dc o", p=P)
    wk = wk_d[:].rearrange("(dc p) o -> p dc o", p=P)
    wv = wv_d[:].rearrange("(dc p) o -> p dc o", p=P)
    out_r = out_d[:].rearrange("(qc p) o -> p qc o", p=P)  # [128, 8, 1024]

    with tile.TileContext(nc, pool_alloc_mode="queue") as tc:
      for _rep in range(repeat):
        with tc.tile_pool(name="resident", bufs=1) as res_pool:
            kt_res = res_pool.tile([P, OC, S], F32R, name="kt_res")
            ones2 = res_pool.tile([P, 2], F32R, name="ones2")
            nc.vector.memset(ones2[:].bitcast(F32), 1.0)

            with tc.tile_pool(name="dram_scratch", bufs=1, space="DRAM") as dpool:
                qt_dram = dpool.tile([P, OC, SLOTS * QB], F32, name="qt_dram")
                v_dram = dpool.tile([P, TC, D], F32, name="v_dram")

                # ---- Phase Q: Q^T = Wq^T.T @ xTq -> qt_dram ----------------
                with (
                    tc.tile_pool(name="wq_pool", bufs=1) as wpool,
                    tc.tile_pool(name="xq_pool", bufs=1) as xpool,
                    tc.tile_pool(name="evq_pool", bufs=3) as evpool,
                    tc.tile_pool(name="psum_q", bufs=4, space="PSUM") as pspool,
                ):
                    w_t = wpool.tile([P, DC, D], F32R, name="wq_t")
                    nc.sync.dma_start(w_t[:], wq.bitcast(F32R))
                    for qt in range(2):  # 512-wide query column tiles
                        x_t = xpool.tile([P, DC, 512], F32R, name="xq_t")
                        nc.sync.dma_start(
                            x_t[:], xTq[:, :, 512 * qt : 512 * (qt + 1)].bitcast(F32R)
                        )
                        for oc in range(OC):
                            ps = pspool.tile([P, 512], F32, name="ps_q")
                            for dc in range(DC):
                                nc.tensor.matmul(
                                    ps[:],
                                    lhsT=w_t[:, dc, P * oc : P * (oc + 1)],
                                    rhs=x_t[:, dc, :],
                                    start=(dc == 0),
                                    stop=(dc == DC - 1),
                                )
                            ev = evpool.tile([P, 512], F32, name="qt_ev")
                            nc.vector.tensor_copy(ev[:], ps[:])
                            nc.sync.dma_start(
                                qt_dram[:, oc, 512 * qt : 512 * (qt + 1)], ev[:]
                            )

                # ---- Phase K: K^T = Wk^T.T @ xT -> kt_res ------------------
                with (
                    tc.tile_pool(name="wk_pool", bufs=1) as wpool,
                    tc.tile_pool(name="xk_pool", bufs=2) as xpool,
                    tc.tile_pool(name="psum_k", bufs=4, space="PSUM") as pspool,
                ):
                    w_t = wpool.tile([P, DC, D], F32R, name="wk_t")
                    nc.sync.dma_start(w_t[:], wk.bitcast(F32R))
                    for tt in range(4):  # 512-wide context column tiles
                        x_t = xpool.tile([P, DC, 512], F32R, name="xk_t")
                        nc.sync.dma_start(
                            x_t[:], xT[:, :, 512 * tt : 512 * (tt + 1)].bitcast(F32R)
                        )
                        for oc in range(OC):
                            ps = pspool.tile([P, 512], F32, name="ps_k")
                            for dc in range(DC):
                                nc.tensor.matmul(
                                    ps[:],
                                    lhsT=w_t[:, dc, P * oc : P * (oc + 1)],
                                    rhs=x_t[:, dc, :],
                                    start=(dc == 0),
                                    stop=(dc == DC - 1),
                                )
                            nc.vector.tensor_copy(
                                kt_res[:, oc, 512 * tt : 512 * (tt + 1)], ps[:]
                            )

                # ---- Phase V: V = xT.T @ Wv^T -> v_dram --------------------
                with (
                    tc.tile_pool(name="wv_pool", bufs=1) as wpool,
                    tc.tile_pool(name="xv_pool", bufs=2) as xpool,
                    tc.tile_pool(name="evv_pool", bufs=3) as evpool,
                    tc.tile_pool(name="psum_v", bufs=4, space="PSUM") as pspool,
                ):
                    w_t = wpool.tile([P, DC, D], F32R, name="wv_t")
                    nc.sync.dma_start(w_t[:], wv.bitcast(F32R))
                    for tt in range(8):  # 256-wide x tiles (feed lhsT only)
                        x_t = xpool.tile([P, DC, 256], F32R, name="xv_t")
                        nc.sync.dma_start(
                            x_t[:], xT[:, :, 256 * tt : 256 * (tt + 1)].bitcast(F32R)
                        )
                        for tci in range(2):
                            tcg = 2 * tt + tci
                            for ot in range(2):
                                ps = pspool.tile([P, 512], F32, name="ps_v")
                                for dc in range(DC):
                                    nc.tensor.matmul(
                                        ps[:],
                                        lhsT=x_t[:, dc, P * tci : P * (tci + 1)],
                                        rhs=w_t[:, dc, 512 * ot : 512 * (ot + 1)],
                                        start=(dc == 0),
                                        stop=(dc == DC - 1),
                                    )
                                ev = evpool.tile([P, 512], F32, name="v_ev")
                                nc.vector.tensor_copy(ev[:], ps[:])
                                nc.sync.dma_start(
                                    v_dram[:, tcg, 512 * ot : 512 * (ot + 1)], ev[:]
                                )

                # ---- Phase A: attention, one slot-pair (512 q) at a time ---
                with (
                    tc.tile_pool(name="qt_pool", bufs=2) as qpool,
                    tc.tile_pool(name="es_pool", bufs=16) as epool,
                    tc.tile_pool(name="vt_pool", bufs=3) as vpool,
                    tc.tile_pool(name="mk_pool", bufs=2) as mpool,
                    tc.tile_pool(name="ob_pool", bufs=3) as opool,
                    tc.tile_pool(name="rc_pool", bufs=2) as rpool,
                    tc.tile_pool(name="psum_s", bufs=2, space="PSUM") as pss,
                    tc.tile_pool(name="psum_o", bufs=4, space="PSUM") as pso_pool,
                    tc.tile_pool(name="psum_d", bufs=2, space="PSUM") as psd_pool,
                ):
                    mask_i = 0
                    for p in range(2):  # slot pairs (0,1), (2,3)
                        depth = PAIR_DEPTH[p]
                        qts = qpool.tile([P, OC, 512], F32R, name="qt_pair")
                        nc.sync.dma_start(
                            qts[:],
                            qt_dram[:, :, 512 * p : 512 * (p + 1)].bitcast(F32R),
                        )
                        # scores + exp + mask for the union depth
                        es_tiles = []
                        for kt in range(depth):
                            ps_s = pss.tile([P, 512], F32, name="ps_s")
                            for oc in range(OC):
                                nc.tensor.matmul(
                                    ps_s[:],
                                    lhsT=kt_res[:, oc, P * kt : P * (kt + 1)],
                                    rhs=qts[:, oc, :],
                                    start=(oc == 0),
                                    stop=(oc == OC - 1),
                                )
                            es = epool.tile([P, 512], F32R, name="es")
                            nc.scalar.activation(es[:], ps_s[:], AF.Exp, scale=SCALE)
                            if p == 1 and kt < 8:
                                pass  # both slots fully valid, no mask needed
                            else:
                                mt = mpool.tile([P, 512], F32R, name="mask_t")
                                nc.sync.dma_start(mt[:], mask_d[mask_i].bitcast(F32R))
                                nc.vector.tensor_mul(out=es[:], in0=es[:], in1=mt[:])
                                mask_i += 1
                            es_tiles.append(es)
                        # attn@V: two sweeps (slot A: qcc 0,1; slot B: qcc 2,3)
                        for sw, qccs in enumerate(((0, 1), (2, 3))):
                            sdepth = KT_COUNTS[2 * p + sw]
                            pso = {
                                (qcc, ot): pso_pool.tile([P, 512], F32, name="ps_o")
                                for qcc in qccs
                                for ot in range(2)
                            }
                            psd = {
                                qcc: psd_pool.tile([P, 2], F32, name="ps_d")
                                for qcc in qccs
                            }
                            for kt in range(sdepth):
                                v_t = vpool.tile([P, D], F32R, name="v_t")
                                nc.sync.dma_start(
                                    v_t[:], v_dram[:, kt, :].bitcast(F32R)
                                )
                                first, last = (kt == 0), (kt == sdepth - 1)
                                for qcc in qccs:
                                    lhs = es_tiles[kt][:, P * qcc : P * (qcc + 1)]
                                    for ot in range(2):
                                        nc.tensor.matmul(
                                            pso[(qcc, ot)][:],
                                            lhsT=lhs,
                                            rhs=v_t[:, 512 * ot : 512 * (ot + 1)],
                                            start=first,
                                            stop=last,
                                        )
                                    nc.tensor.matmul(
                                        psd[qcc][:],
                                        lhsT=lhs,
                                        rhs=ones2[:],
                                        start=first,
                                        stop=last,
                                    )
                            for qcc in qccs:
                                rc = rpool.tile([P, 1], F32, name="rc")
                                nc.vector.reciprocal(rc[:], psd[qcc][:, 0:1])
                                for ot in range(2):
                                    ob = opool.tile([P, 512], F32, name="ob")
                                    nc.scalar.activation(
                                        ob[:], pso[(qcc, ot)][:], AF.Copy, scale=rc[:]
                                    )
                                    nc.sync.dma_start(
                                        out_r[
                                            :, 4 * p + qcc, 512 * ot : 512 * (ot + 1)
                                        ],
                                        ob[:],
                                    )

    nc.compile()
    if not nc.is_finalized():
        nc.finalize()
    return nc


def _build_masks(fold: int) -> np.ndarray:
    """0/1 masks [N_MASK, 128, 512]; cols 0:256 = slot 2p, 256:512 = slot 2p+1."""
    tiles = []
    ki = np.arange(P)[:, None]
    qi = np.arange(QB)[None, :]
    for p in range(2):
        lo = 8 if p == 1 else 0  # pair1 kt<8 is fully valid for both folds
        for kt in range(lo, PAIR_DEPTH[p]):
            k0 = kt * P
            halves = []
            for s in (2 * p, 2 * p + 1):
                q0 = FOLD_QBLOCKS[fold][s] * QB
                halves.append(((q0 + qi) >= (k0 + ki)).astype(np.float32))
            tiles.append(np.concatenate(halves, axis=1))
    return np.ascontiguousarray(np.stack(tiles))


def kernel(**inputs: np.ndarray) -> np.ndarray:
    x = np.ascontiguousarray(np.asarray(inputs["inputs"], dtype=np.float32))
    wqT = np.ascontiguousarray(np.asarray(inputs["Wq"], dtype=np.float32).T)
    wkT = np.ascontiguousarray(np.asarray(inputs["Wk"], dtype=np.float32).T)
    wvT = np.ascontiguousarray(np.asarray(inputs["Wv"], dtype=np.float32).T)

    masks = {f: _build_masks(f) for f in (0, 1)}
    in_maps = []
    for c in range(N_CORES):
        b, f = c // 2, c % 2
        xT = np.ascontiguousarray(x[b].T)  # [D, S]
        xTq = np.ascontiguousarray(
            np.concatenate(
                [xT[:, qb * QB : (qb + 1) * QB] for qb in FOLD_QBLOCKS[f]], axis=1
            )
        )
        in_maps.append(
            {
                "xT": xT,
                "xTq": xTq,
                "wqT": wqT,
                "wkT": wkT,
                "wvT": wvT,
                "masks": masks[f],
            }
        )

    nc = _build_nc()
    res = run_bass_kernel_spmd(nc, in_maps, core_ids=list(range(N_CORES)))

    out = np.empty((B, S, D), dtype=np.float32)
    for c in range(N_CORES):
        b, f = c // 2, c % 2
        o = res.results[c]["out"]  # [1024, 1024] rows in slot order
        for s, qb in enumerate(FOLD_QBLOCKS[f]):
            out[b, qb * QB : (qb + 1) * QB, :] = o[s * QB : (s + 1) * QB, :]
    return out


# revision 5
# speedup vs baseline: 37.2308x; 1.3544x over previous
"""Causal attention (B=4, S=2048, D=1024) on 8 Trainium2 NeuronCores.

Sharding: 2 cores per batch element. Within a batch, the 8 query blocks of
256 rows are split between the two cores by parity (fold 0 takes odd blocks,
fold 1 takes even blocks) so causal-attention work is balanced. Each core
computes Q for its own 1024 query rows, and K/V for the full 2048 context
rows (duplicated across the pair — cheaper than a collective here).

All matmuls run in bf16 (fp32 accumulate in PSUM) with N=512 moving operands:
the PE streams one 128x128x512 matmul every ~215 ns with the bf16
fast-weight-load fully hidden, and bf16 halves all DMA traffic and SBUF
footprints (K^T, V, Q^T, and exp(S) all stay resident / tiny). End-to-end
absmax-relative error vs the fp32 reference is ~3e-3.

Layout trick: scores are computed transposed (k on partitions, q on free dim)
via S^T = K^T.T @ Q^T, so no transpose of the softmax matrix is needed:
exp(S^T) tiles feed attn@V directly as the stationary operand, producing the
output in natural [q, o] layout. Scores for two adjacent 256-row query slots
are computed together (N=512) over the union of their causal depths; the 0/1
causal masks (streamed per-core from the host, so one SPMD program serves
both folds) zero both the diagonal parts and the over-computed region, which
also keeps the softmax denominators correct. Softmax skips max-subtraction
(scores/sqrt(d) are ~N(0,1) here; exp cannot overflow), with denominators
from a ones-column matmul per 128-query chunk.
"""

import sys

sys.path.insert(0, "/opt/trn_rl_repo")

import ml_dtypes
import numpy as np

import concourse.bass as bass  # noqa: F401
import concourse.mybir as mybir
import concourse.tile as tile
from concourse import bacc
from concourse.bass_utils import run_bass_kernel_spmd

F32 = mybir.dt.float32
BF16 = mybir.dt.bfloat16
AF = mybir.ActivationFunctionType

B, S, D = 4, 2048, 1024
P = 128
DC = D // P  # 8 contraction chunks
OC = D // P  # 8 output-feature chunks
TC = S // P  # 16 context chunks
N_CORES = 8
SLOTS = 4  # query slots of 256 rows per core
QB = 256
# Padded causal depth (in 128-wide k tiles) per slot, fold-uniform:
# fold 0 owns global 256-blocks [1,3,5,7] (true depths 4,8,12,16),
# fold 1 owns [0,2,4,6] (true depths 2,6,10,14) -> padded to fold-0 depths.
KT_COUNTS = [4, 8, 12, 16]
FOLD_QBLOCKS = {0: [1, 3, 5, 7], 1: [0, 2, 4, 6]}
# Slot pairs (0,1) and (2,3) share one N=512 scores pass over the union depth.
PAIR_DEPTH = [KT_COUNTS[1], KT_COUNTS[3]]  # [8, 16]
N_MASK = sum(PAIR_DEPTH) - 8  # pair0: kt 0..7 masked; pair1: kt 8..15 masked
SCALE = 1.0 / np.sqrt(np.float32(D))


def _build_nc(repeat: int = 1):
    nc = bacc.Bacc("TRN2", target_bir_lowering=False, debug=False, num_devices=N_CORES)

    xT_d = nc.declare_dram_parameter("xT", [D, S], BF16, isOutput=False)
    xTq_d = nc.declare_dram_parameter("xTq", [D, SLOTS * QB], BF16, isOutput=False)
    wq_d = nc.declare_dram_parameter("wqT", [D, D], BF16, isOutput=False)
    wk_d = nc.declare_dram_parameter("wkT", [D, D], BF16, isOutput=False)
    wv_d = nc.declare_dram_parameter("wvT", [D, D], BF16, isOutput=False)
    mask_d = nc.declare_dram_parameter(
        "masks", [N_MASK, P, 2 * QB], BF16, isOutput=False
    )
    out_d = nc.declare_dram_parameter("out", [SLOTS * QB, D], F32, isOutput=True)

    xT = xT_d[:].rearrange("(dc p) t -> p dc t", p=P)  # [128, 8, 2048]
    xTq = xTq_d[:].rearrange("(dc p) q -> p dc q", p=P)  # [128, 8, 1024]
    wq = wq_d[:].rearrange("(dc p) o -> p dc o", p=P)
    wk = wk_d[:].rearrange("(dc p) o -> p dc o", p=P)
    wv = wv_d[:].rearrange("(dc p) o -> p dc o", p=P)
    out_r = out_d[:].rearrange("(qc p) o -> p qc o", p=P)  # [128, 8, 1024]

    with tile.TileContext(nc, pool_alloc_mode="queue") as tc:
      for _rep in range(repeat):
        with tc.tile_pool(name="resident", bufs=1) as res_pool:
            kt_res = res_pool.tile([P, OC, S], BF16, name="kt_res")
            v_res = res_pool.tile([P, TC, D], BF16, name="v_res")
            qt_res = res_pool.tile([P, OC, SLOTS * QB], BF16, name="qt_res")
            ones2 = res_pool.tile([P, 2], BF16, name="ones2")
            nc.vector.memset(ones2[:], 1.0)

            # ---- Phase Q: Q^T = Wq^T.T @ xTq -> qt_res (SBUF) --------------
            with (
                tc.tile_pool(name="wq_pool", bufs=1) as wpool,
                tc.tile_pool(name="xq_pool", bufs=2) as xpool,
                tc.tile_pool(name="psum_q", bufs=4, space="PSUM") as pspool,
            ):
                w_t = wpool.tile([P, DC, D], BF16, name="wq_t")
                for dc in range(DC):  # chunked so first matmuls start early
                    nc.sync.dma_start(w_t[:, dc, :], wq[:, dc, :])
                for qt in range(2):  # 512-wide query column tiles
                    x_t = xpool.tile([P, DC, 512], BF16, name="xq_t")
                    nc.sync.dma_start(x_t[:], xTq[:, :, 512 * qt : 512 * (qt + 1)])
                    for oc in range(OC):
                        ps = pspool.tile([P, 512], F32, name="ps_q")
                        for dc in range(DC):
                            nc.tensor.matmul(
                                ps[:],
                                lhsT=w_t[:, dc, P * oc : P * (oc + 1)],
                                rhs=x_t[:, dc, :],
                                start=(dc == 0),
                                stop=(dc == DC - 1),
                            )
                        nc.vector.tensor_copy(
                            qt_res[:, oc, 512 * qt : 512 * (qt + 1)], ps[:]
                        )

            # ---- Phase KV (merged, one pass over xT): K^T and V ------------
            with (
                tc.tile_pool(name="wk_pool", bufs=1) as wkpool,
                tc.tile_pool(name="wv_pool", bufs=1) as wvpool,
                tc.tile_pool(name="xkv_pool", bufs=2) as xpool,
                tc.tile_pool(name="psum_kv", bufs=6, space="PSUM") as pspool,
            ):
                wk_t = wkpool.tile([P, DC, D], BF16, name="wk_t")
                wv_t = wvpool.tile([P, DC, D], BF16, name="wv_t")
                for dc in range(DC):
                    nc.sync.dma_start(wk_t[:, dc, :], wk[:, dc, :])
                    nc.sync.dma_start(wv_t[:, dc, :], wv[:, dc, :])
                for tt in range(4):  # 512-wide context tiles
                    x_t = xpool.tile([P, DC, 512], BF16, name="xkv_t")
                    nc.sync.dma_start(x_t[:], xT[:, :, 512 * tt : 512 * (tt + 1)])
                    # K^T: [o-part, t]
                    for oc in range(OC):
                        ps = pspool.tile([P, 512], F32, name="ps_k", tag="ps_kv")
                        for dc in range(DC):
                            nc.tensor.matmul(
                                ps[:],
                                lhsT=wk_t[:, dc, P * oc : P * (oc + 1)],
                                rhs=x_t[:, dc, :],
                                start=(dc == 0),
                                stop=(dc == DC - 1),
                            )
                        nc.vector.tensor_copy(
                            kt_res[:, oc, 512 * tt : 512 * (tt + 1)], ps[:]
                        )
                    # V: [t-part, o]
                    for tci in range(4):
                        tcg = 4 * tt + tci
                        for ot in range(2):
                            ps = pspool.tile([P, 512], F32, name="ps_v", tag="ps_kv")
                            for dc in range(DC):
                                nc.tensor.matmul(
                                    ps[:],
                                    lhsT=x_t[:, dc, P * tci : P * (tci + 1)],
                                    rhs=wv_t[:, dc, 512 * ot : 512 * (ot + 1)],
                                    start=(dc == 0),
                                    stop=(dc == DC - 1),
                                )
                            nc.vector.tensor_copy(
                                v_res[:, tcg, 512 * ot : 512 * (ot + 1)], ps[:]
                            )

            # ---- Phase A: attention, one slot-pair (512 q) at a time -------
            with (
                tc.tile_pool(name="es_pool", bufs=16) as epool,
                tc.tile_pool(name="mk_pool", bufs=2) as mpool,
                tc.tile_pool(name="ob_pool", bufs=3) as opool,
                tc.tile_pool(name="rc_pool", bufs=2) as rpool,
                tc.tile_pool(name="psum_s", bufs=2, space="PSUM") as pss,
                tc.tile_pool(name="psum_o", bufs=4, space="PSUM") as pso_pool,
                tc.tile_pool(name="psum_d", bufs=2, space="PSUM") as psd_pool,
            ):
                mask_i = 0
                for p in range(2):  # slot pairs (0,1), (2,3)
                    depth = PAIR_DEPTH[p]
                    # scores + exp + mask over the union depth
                    es_tiles = []
                    for kt in range(depth):
                        ps_s = pss.tile([P, 512], F32, name="ps_s")
                        for oc in range(OC):
                            nc.tensor.matmul(
                                ps_s[:],
                                lhsT=kt_res[:, oc, P * kt : P * (kt + 1)],
                                rhs=qt_res[:, oc, 512 * p : 512 * (p + 1)],
                                start=(oc == 0),
                                stop=(oc == OC - 1),
                            )
                        es = epool.tile([P, 512], BF16, name="es")
                        nc.scalar.activation(es[:], ps_s[:], AF.Exp, scale=SCALE)
                        if p == 1 and kt < 8:
                            pass  # both slots fully valid, no mask needed
                        else:
                            mt = mpool.tile([P, 512], BF16, name="mask_t")
                            nc.sync.dma_start(mt[:], mask_d[mask_i])
                            nc.vector.tensor_mul(out=es[:], in0=es[:], in1=mt[:])
                            mask_i += 1
                        es_tiles.append(es)
                    # attn@V: two sweeps (slot A: qcc 0,1; slot B: qcc 2,3)
                    for sw, qccs in enumerate(((0, 1), (2, 3))):
                        sdepth = KT_COUNTS[2 * p + sw]
                        pso = {
                            (qcc, ot): pso_pool.tile([P, 512], F32, name="ps_o")
                            for qcc in qccs
                            for ot in range(2)
                        }
                        psd = {
                            qcc: psd_pool.tile([P, 2], F32, name="ps_d")
                            for qcc in qccs
                        }
                        for kt in range(sdepth):
                            first, last = (kt == 0), (kt == sdepth - 1)
                            for qcc in qccs:
                                lhs = es_tiles[kt][:, P * qcc : P * (qcc + 1)]
                                for ot in range(2):
                                    nc.tensor.matmul(
                                        pso[(qcc, ot)][:],
                                        lhsT=lhs,
                                        rhs=v_res[:, kt, 512 * ot : 512 * (ot + 1)],
                                        start=first,
                                        stop=last,
                                    )
                                nc.tensor.matmul(
                                    psd[qcc][:],
                                    lhsT=lhs,
                                    rhs=ones2[:],
                                    start=first,
                                    stop=last,
                                )
                        for qcc in qccs:
                            rc = rpool.tile([P, 1], F32, name="rc")
                            nc.vector.reciprocal(rc[:], psd[qcc][:, 0:1])
                            for ot in range(2):
                                ob = opool.tile([P, 512], F32, name="ob")
                                nc.scalar.activation(
                                    ob[:], pso[(qcc, ot)][:], AF.Copy, scale=rc[:]
                                )
                                nc.sync.dma_start(
                                    out_r[:, 4 * p + qcc, 512 * ot : 512 * (ot + 1)],
                                    ob[:],
                                )

    nc.compile()
    if not nc.is_finalized():
        nc.finalize()
    return nc


def _build_masks(fold: int) -> np.ndarray:
    """0/1 masks [N_MASK, 128, 512]; cols 0:256 = slot 2p, 256:512 = slot 2p+1."""
    tiles = []
    ki = np.arange(P)[:, None]
    qi = np.arange(QB)[None, :]
    for p in range(2):
        lo = 8 if p == 1 else 0  # pair1 kt<8 is fully valid for both folds
        for kt in range(lo, PAIR_DEPTH[p]):
            k0 = kt * P
            halves = []
            for s in (2 * p, 2 * p + 1):
                q0 = FOLD_QBLOCKS[fold][s] * QB
                halves.append(((q0 + qi) >= (k0 + ki)).astype(np.float32))
            tiles.append(np.concatenate(halves, axis=1))
    return np.ascontiguousarray(np.stack(tiles).astype(ml_dtypes.bfloat16))


def kernel(**inputs: np.ndarray) -> np.ndarray:
    x = np.asarray(inputs["inputs"], dtype=np.float32)
    bf = ml_dtypes.bfloat16
    wqT = np.ascontiguousarray(np.asarray(inputs["Wq"], dtype=np.float32).T.astype(bf))
    wkT = np.ascontiguousarray(np.asarray(inputs["Wk"], dtype=np.float32).T.astype(bf))
    wvT = np.ascontiguousarray(np.asarray(inputs["Wv"], dtype=np.float32).T.astype(bf))

    masks = {f: _build_masks(f) for f in (0, 1)}
    in_maps = []
    for c in range(N_CORES):
        b, f = c // 2, c % 2
        xT = np.ascontiguousarray(x[b].T.astype(bf))  # [D, S]
        xTq = np.ascontiguousarray(
            np.concatenate(
                [xT[:, qb * QB : (qb + 1) * QB] for qb in FOLD_QBLOCKS[f]], axis=1
            )
        )
        in_maps.append(
            {
                "xT": xT,
                "xTq": xTq,
                "wqT": wqT,
                "wkT": wkT,
                "wvT": wvT,
                "masks": masks[f],
            }
        )

    nc = _build_nc()
    res = run_bass_kernel_spmd(nc, in_maps, core_ids=list(range(N_CORES)))

    out = np.empty((B, S, D), dtype=np.float32)
    for c in range(N_CORES):
        b, f = c // 2, c % 2
        o = res.results[c]["out"]  # [1024, 1024] rows in slot order
        for s, qb in enumerate(FOLD_QBLOCKS[f]):
            out[b, qb * QB : (qb + 1) * QB, :] = o[s * QB : (s + 1) * QB, :]
    return out
